# revision 54
# baseline (speedup 1.0000x reference)
"""Causal self-attention on 8 Trainium2 NeuronCores (Bass/Tile).

Problem: x[4,2048,1024] @ W_attn[1024,3072] + b_attn -> qkv; 16-head causal
attention; y @ W_proj[1024,1024] + b_proj.

Sharding: 2D over (batch, head-group), zero inter-core communication.
Core c = (b = c//2, g = c%2); each core computes q/k/v for its 8 heads over
its batch, flash-style causal attention (no max subtraction -- logits are
small -- with the softmax denominator accumulated as a 65th "ones" column
of v), then a partial output projection with its 512-row slice of W_proj.
The host adds the two partials per batch plus b_proj and the v-bias
projection bv @ W_proj (softmax weights sum to 1, so the v-bias passes
through attention unchanged and never needs to enter the kernel).

Kernel structure (build_nc2), tuned against TimelineSim + HW runs:
  - chunk-pipelined emission: per 512-token chunk c, stage B (transposed
    x load + qkv projection), deferred stage-D units (output projection),
    stage C
    (attention over key blocks <= c). Per-engine queues are in-order, so
    interleaving emission is what lets ACT's exp overlap PE's projection
    matmuls (serial-phase version was ACT-bound at 99% during attention).
  - all matmul operands bf16 (fp32 PSUM accumulation); inputs and output
    cast host-side; rel err ~3.6e-3 vs fp32 reference (tolerance 2e-2).
  - resident weights: wq/wk/wv/wp loaded to SBUF once (~3.5MB total);
    chunk-0 x tiles prefetched ahead of them; per-core DMA is 11.5MB total
    vs 42MB for the per-chunk-reload fp32 version.
  - head-paired attention: heads (2i, 2i+1) sit at partitions 0-63/64-127
    of feature block i; their K=64 QK matmuls are emitted adjacently with
    row tile positions (0,0)/(64,0) so the PE array runs both concurrently
    (2x QK throughput on HW; not modeled by TimelineSim). Their score
    regions are padded to separate PSUM banks -- concurrent matmuls must
    not share a 2KB bank (same-bank variant faults on HW).
  - software-pipelined inner loop: QK(j+1) is emitted before AV(j) so the
    PE computes the next block's scores while ACT runs exp on the current
    one; exp covers both heads' scores in one [128, 2*TCH] instruction.
  - causal trimming: diagonal key blocks stream only the valid query
    columns (N = TCH-128r) through QK, exp, mask and AV.
  - deferred output projection: D units are drip-fed between attention
    pairs as PE filler where ACT is the local bottleneck (last chunk).

TimelineSim: 270us vs 451us for the previous kernel; PE ~88% busy, ACT 147us,
DVE 122us, DMA 38us. x is loaded pre-transposed via the DMA XBAR
(dma_start_transpose), removing all PE transposes and their DVE copies. HW reps-slope measurements (axon tunnel, contended):
0.2-0.7ms per body vs ~1.0ms for the previous kernel.
"""

import numpy as np

import concourse.mybir as mybir
import concourse.tile as tile
from concourse import bacc
from concourse.bass_utils import run_bass_kernel_spmd

F32 = mybir.dt.float32
F32R = mybir.dt.float32r
BF16 = mybir.dt.bfloat16

B, T, D, H = 4, 2048, 1024, 16
HD = D // H               # 64
N_GROUPS = 2
FQ = D // N_GROUPS        # 512 features (8 heads) per core
N_CORES = B * N_GROUPS

# set by test harness to collect an NTFF trace / HW exec time
TRACE = False
LAST_RESULTS = None


def build_nc2(reps=1, x_dt=BF16, qk_dt=BF16, av_dt=BF16, pj_dt=BF16,
              ps_bufs=2, py_bufs=1, pj_bufs=2, est_bufs=3,
              out_bufs=4, trim=True, out_dt=BF16, TCH=512, small_bufs=3):
    """Chunk-pipelined causal attention kernel for one core (8 heads, one
    batch's 2048 tokens, feature half [g*512:(g+1)*512])."""
    P = 128
    NTC = T // TCH            # token chunks
    TBC = TCH // P            # 4 token blocks per chunk
    DCH = D // P              # 8 contraction blocks
    NFB = FQ // P             # 4 feature blocks of qT/kT
    HLOC = FQ // HD           # 8 heads on this core
    NPAIR = HLOC // 2         # 4 head pairs
    NTB = T // P              # 16 key blocks total
    NLC = FQ // P             # 4 feature blocks of y
    DOUT_CH = 512
    NDOUT = D // DOUT_CH      # 2
    scale = 1.0 / float(np.sqrt(HD))

    nc = bacc.Bacc()
    xb = nc.dram_tensor("xb", [T, D], x_dt, kind="ExternalInput")
    wq = nc.dram_tensor("wq", [D, FQ], x_dt, kind="ExternalInput")
    wk = nc.dram_tensor("wk", [D, FQ], x_dt, kind="ExternalInput")
    wv = nc.dram_tensor("wv", [D, FQ], x_dt, kind="ExternalInput")
    bq = nc.dram_tensor("bq", [FQ], F32, kind="ExternalInput")
    bk = nc.dram_tensor("bk", [FQ], F32, kind="ExternalInput")
    wp = nc.dram_tensor("wp", [FQ, D], pj_dt, kind="ExternalInput")
    out = nc.dram_tensor("out", [T, D], out_dt, kind="ExternalOutput")

    with tile.TileContext(nc) as tc:
        with (
            tc.tile_pool(name="const", bufs=1) as const,
            tc.tile_pool(name="big", bufs=1) as big,
            tc.tile_pool(name="xtp", bufs=2) as xtp,
            tc.tile_pool(name="est", bufs=est_bufs) as est,
            tc.tile_pool(name="small", bufs=small_bufs) as small,
            tc.tile_pool(name="outp", bufs=out_bufs) as outp,
            tc.tile_pool(name="ps", bufs=ps_bufs, space="PSUM") as ps,
            tc.tile_pool(name="pj", bufs=pj_bufs, space="PSUM") as pj,
            tc.tile_pool(name="py", bufs=py_bufs, space="PSUM") as pyp,
        ):
            # diagonal-block masks: mask_r[p, f] = 1 if f >= p + 128*r else 0
            masks = []
            for r in range(TBC):
                m = const.tile([P, TCH], av_dt, tag=f"mask{r}")
                nc.gpsimd.memset(m, 1.0)
                nc.gpsimd.affine_select(
                    out=m, in_=m,
                    compare_op=mybir.AluOpType.is_ge,
                    fill=0.0,
                    base=-P * r,
                    pattern=[[1, TCH]],
                    channel_multiplier=-1,
                )
                masks.append(m)
            # startup-critical DMA order (the DMA engines drain mostly
            # serially): xT block 0, then wv, then the rest of chunk-0's xT
            # -- the first v-projection needs only xT(tb0)+wv, so PE work
            # starts after ~4us instead of ~12
            xpre = xtp.tile([P, DCH, TCH], x_dt, tag="xT")
            nc.sync.dma_start_transpose(
                out=xpre[:, :, 0:P], in_=xb[0:P, :])
            wv_sb = big.tile([P, DCH, FQ], x_dt, tag="wv_sb")
            nc.sync.dma_start(out=wv_sb, in_=wv.rearrange("(dc p) f -> p dc f", p=P))
            for tb in range(1, TCH // P):
                nc.sync.dma_start_transpose(
                    out=xpre[:, :, tb * P:(tb + 1) * P],
                    in_=xb[tb * P:(tb + 1) * P, :])
            wq_sb = big.tile([P, DCH, FQ], x_dt, tag="wq_sb")
            wk_sb = big.tile([P, DCH, FQ], x_dt, tag="wk_sb")
            for (w_sb, w_dram) in ((wq_sb, wq), (wk_sb, wk)):
                for fh in range(2):
                    f0, f1 = fh * (FQ // 2), (fh + 1) * (FQ // 2)
                    nc.sync.dma_start(
                        out=w_sb[:, :, f0:f1],
                        in_=w_dram.rearrange("(dc p) f -> p dc f", p=P)[:, :, f0:f1])
            # biases after the bulk weights: tiny but descriptor-heavy DMAs
            bq_sb = const.tile([P, NFB], F32, tag="bq")
            nc.sync.dma_start(out=bq_sb, in_=bq.rearrange("(o p) -> p o", p=P))
            bk_sb = const.tile([P, NFB], F32, tag="bk")
            nc.sync.dma_start(out=bk_sb, in_=bk.rearrange("(o p) -> p o", p=P))
            # wp_sb[:, lc, o, :] = wp[lc*128:(lc+1)*128, o*512:(o+1)*512]
            wp_sb = big.tile([P, NLC, NDOUT, DOUT_CH], pj_dt, tag="wp_sb")
            nc.sync.dma_start(
                out=wp_sb,
                in_=wp.rearrange("(lc p) (o q) -> p lc o q", p=P, q=DOUT_CH))

            for _rep in range(reps):
                qT = big.tile([P, NFB, T], qk_dt, tag="qT")     # [f%128, fb, tok]
                kT = big.tile([P, NFB, T], qk_dt, tag="kT")
                v_aug = big.tile([P, NTB, HLOC, HD + 1], av_dt, tag="v")
                # per-lc yT tiles keep the output projection's dependency
                # on each head pair independent (no whole-tile coupling)
                yTs = [big.tile([P, T], pj_dt, tag=f"yT{lc}", name=f"yT{lc}")
                       for lc in range(NLC)]

                nc.vector.memset(v_aug[:, :, :, HD:HD + 1], 1.0)

                def stage_b(c, xtiles=None):
                    """load x chunk transposed via the DMA XBAR, project q/k
                    (-> [f, tok]) and v."""
                    t0 = c * TCH
                    if xtiles is not None:
                        xT = xtiles
                    else:
                        xT = xtp.tile([P, DCH, TCH], x_dt, tag="xT")
                        nc.sync.dma_start_transpose(
                            out=xT, in_=xb[t0:t0 + TCH, :])
                    for tb in range(TBC):
                        # v carries no bias: softmax weights sum to 1, so
                        # the v-bias passes through attention unchanged and
                        # its projection bv @ W_proj is added on the host
                        pv = pj.tile([P, 512], F32, tag="pj")
                        for d in range(DCH):
                            nc.tensor.matmul(
                                pv[:, :FQ],
                                xT[:, d, tb * P:(tb + 1) * P],
                                wv_sb[:, d, :],
                                start=(d == 0), stop=(d == DCH - 1),
                            )
                        nc.vector.tensor_copy(
                            out=v_aug[:, c * TBC + tb, :, 0:HD],
                            in_=pv[:, :FQ].rearrange("p (h d) -> p h d", d=HD),
                        )
                    for (w_sb, bias_sb, dstT) in (
                            (wq_sb, bq_sb, qT), (wk_sb, bk_sb, kT)):
                        for fb in range(NFB):
                            pq = pj.tile([P, TCH], F32, tag="pj")
                            for d in range(DCH):
                                nc.tensor.matmul(
                                    pq[:, :TCH],
                                    w_sb[:, d, fb * P:(fb + 1) * P],
                                    xT[:, d, :],
                                    start=(d == 0), stop=(d == DCH - 1),
                                )
                            nc.vector.tensor_scalar_add(
                                out=dstT[:, fb, t0:t0 + TCH], in0=pq[:, :TCH],
                                scalar1=bias_sb[:, fb:fb + 1],
                            )

                def stage_c_pair(c, hp):
                    """causal attention for query chunk c, head pair hp.

                    Software-pipelined: QK(j+1) is emitted before AV(j) so the
                    in-order PE queue computes the next block's scores while
                    ACT runs exp on the current block."""
                    q0 = c * TCH
                    nj = TBC * c + TBC

                    def q_off_of(j):
                        r = j - TBC * c
                        return 0 if (r < 0 or not trim) else P * r

                    # head B's score region starts at a PSUM bank boundary:
                    # the paired QK matmuls execute concurrently (row tiles)
                    # and must not write the same 2KB bank
                    SCB = max(TCH, 512)

                    def emit_qk(j):
                        q_off = q_off_of(j)
                        sc = ps.tile([P, 2 * SCB], F32, tag="sc")
                        # paired QK: row groups (0,0) and (64,0) run
                        # concurrently on the PE array
                        nc.tensor.matmul(
                            sc[:, q_off:TCH],
                            kT[0:HD, hp, j * P:(j + 1) * P],
                            qT[0:HD, hp, q0 + q_off:q0 + TCH],
                            start=True, stop=True,
                        )
                        nc.tensor.matmul(
                            sc[:, SCB + q_off:SCB + TCH],
                            kT[HD:P, hp, j * P:(j + 1) * P],
                            qT[HD:P, hp, q0 + q_off:q0 + TCH],
                            start=True, stop=True,
                        )
                        e = est.tile([P, 2 * TCH], av_dt, tag="est")
                        sc_v = sc.rearrange(
                            "p (t q) -> p t q", q=SCB)[:, :, q_off:TCH]
                        e_v = e.rearrange(
                            "p (t q) -> p t q", t=2)[:, :, q_off:TCH]
                        nc.scalar.activation(
                            out=e_v, in_=sc_v,
                            func=mybir.ActivationFunctionType.Exp,
                            scale=scale,
                        )
                        r = j - TBC * c
                        if r >= 0:
                            w = TCH - q_off
                            e_2v = e.rearrange(
                                "p (t q) -> p t q", t=2)[:, :, q_off:TCH]
                            m_2v = masks[r][:, q_off:TCH].rearrange(
                                "p (o w) -> p o w", o=1).broadcast_to(
                                [P, 2, w])
                            nc.vector.tensor_mul(out=e_2v, in0=e_2v, in1=m_2v)
                        return e

                    fb = hp
                    py = pyp.tile([P, 2 * TCH], F32, tag="py")
                    e_prev = emit_qk(0)
                    for j in range(nj):
                        e_cur = e_prev
                        if j + 1 < nj:
                            e_prev = emit_qk(j + 1)
                        q_off = q_off_of(j)
                        # when both heads' halves share one 2KB PSUM bank
                        # (TCH<=256), the accumulation group must have a
                        # single start (zeroes the whole bank) and a single
                        # stop; in separate banks each half runs its own group
                        same_bank = (2 * TCH * 4) <= 2048
                        nc.tensor.matmul(
                            py[:HD + 1, q_off:TCH],
                            v_aug[:, j, 2 * hp, :],
                            e_cur[:, q_off:TCH],
                            start=(j == 0),
                            stop=(j == nj - 1) and not same_bank,
                        )
                        nc.tensor.matmul(
                            py[:HD + 1, TCH + q_off:2 * TCH],
                            v_aug[:, j, 2 * hp + 1, :],
                            e_cur[:, TCH + q_off:2 * TCH],
                            start=(j == 0) and not same_bank,
                            stop=(j == nj - 1),
                        )
                    recip = small.tile([1, 2 * TCH], F32, tag="recip")
                    nc.vector.reciprocal(out=recip, in_=py[HD:HD + 1, :])
                    bcA = small.tile([HD, TCH], F32, tag="bcA")
                    nc.gpsimd.partition_broadcast(bcA, recip[:, 0:TCH])
                    bcB = small.tile([HD, TCH], F32, tag="bcB")
                    nc.gpsimd.partition_broadcast(bcB, recip[:, TCH:2 * TCH])
                    nc.vector.tensor_mul(
                        out=yTs[fb][0:HD, q0:q0 + TCH],
                        in0=py[:HD, 0:TCH], in1=bcA)
                    nc.vector.tensor_mul(
                        out=yTs[fb][HD:P, q0:q0 + TCH],
                        in0=py[:HD, TCH:2 * TCH], in1=bcB)

                def d_unit(tbg, o):
                    """one output-projection tile: tokens [tbg*128, +128),
                    output features [o*512, +512)."""
                    po = pj.tile([P, 512], F32, tag="pj")
                    for lc in range(NLC):
                        nc.tensor.matmul(
                            po[:, :DOUT_CH],
                            yTs[lc][:, tbg * P:(tbg + 1) * P],
                            wp_sb[:, lc, o, :],
                            start=(lc == 0), stop=(lc == NLC - 1),
                        )
                    ot = outp.tile([P, DOUT_CH], out_dt, tag="out")
                    nc.vector.tensor_copy(out=ot, in_=po[:, :DOUT_CH])
                    nc.sync.dma_start(
                        out=out[tbg * P:(tbg + 1) * P,
                                o * DOUT_CH:(o + 1) * DOUT_CH],
                        in_=ot,
                    )

                # deferred output-projection queue: D units become ready once
                # their chunk's attention is done; they are drip-fed between
                # attention pairs as PE filler while ACT works through exp
                ready_d = []

                def emit_d(n):
                    for _ in range(min(n, len(ready_d))):
                        d_unit(*ready_d.pop(0))

                dpb = max(1, TBC // 2)
                for c in range(NTC):
                    stage_b(c, xtiles=xpre if (c == 0 and _rep == 0) else None)
                    emit_d(dpb if c < NTC - 1 else 0)
                    for hp in range(NPAIR):
                        stage_c_pair(c, hp)
                        emit_d((1 if c < NTC - 1 else 3) * TBC // 4
                               if TBC >= 4 else (0 if c < NTC - 1 else 2))
                    ready_d += [(c * TBC + tb, o)
                                for tb in range(TBC) for o in range(NDOUT)]
                emit_d(len(ready_d))

    nc.finalize()
    return nc


DEFAULT_CFG = dict(est_bufs=4)

_NC_CACHE = {}


def _get_nc():
    if "nc" not in _NC_CACHE:
        _NC_CACHE["nc"] = build_nc2(**DEFAULT_CFG)
    return _NC_CACHE["nc"]


def _core_inputs(inputs, x_bf16=True, pj_bf16=True):
    import ml_dtypes
    bf = ml_dtypes.bfloat16
    xdt = bf if x_bf16 else np.float32
    pdt = bf if pj_bf16 else np.float32
    x = np.ascontiguousarray(np.asarray(inputs["x"], dtype=np.float32))
    W = np.asarray(inputs["W_attn"], dtype=np.float32)
    ba = np.asarray(inputs["b_attn"], dtype=np.float32)
    Wp = np.asarray(inputs["W_proj"], dtype=np.float32)
    maps = []
    for c in range(N_CORES):
        b, g = c // N_GROUPS, c % N_GROUPS
        s = slice(g * FQ, (g + 1) * FQ)
        maps.append({
            "xb": np.ascontiguousarray(x[b]).astype(xdt),
            "wq": np.ascontiguousarray(W[:, 0:D][:, s]).astype(xdt),
            "wk": np.ascontiguousarray(W[:, D:2 * D][:, s]).astype(xdt),
            "wv": np.ascontiguousarray(W[:, 2 * D:3 * D][:, s]).astype(xdt),
            "bq": np.ascontiguousarray(ba[0:D][s]),
            "bk": np.ascontiguousarray(ba[D:2 * D][s]),
            "wp": np.ascontiguousarray(Wp[s, :]).astype(pdt),
        })
    return maps


def kernel(**inputs) -> np.ndarray:
    global LAST_RESULTS
    nc = _get_nc()
    maps = _core_inputs(inputs)
    res = run_bass_kernel_spmd(
        nc, maps, list(range(N_CORES)), trace=TRACE,
        trace_cores=list(range(N_CORES)) if TRACE else None,
    )
    LAST_RESULTS = res
    bp = np.asarray(inputs["b_proj"], dtype=np.float32)
    # v-bias contribution, exact in f32: bv @ W_proj (see kernel docstring)
    bv = np.asarray(inputs["b_attn"], dtype=np.float32)[2 * D:3 * D]
    bvp = bv @ np.asarray(inputs["W_proj"], dtype=np.float32)
    out = np.empty((B, T, D), dtype=np.float32)
    for b in range(B):
        acc = res.results[b * N_GROUPS]["out"].astype(np.float32).copy()
        for g in range(1, N_GROUPS):
            acc += res.results[b * N_GROUPS + g]["out"]
        out[b] = acc + bp + bvp
    return out

